# revision 57
# baseline (speedup 1.0000x reference)
"""Trainium2 Bass kernel for nn_EqModelComplex (complex-valued pre-LN transformer
block: complex LN -> complex QKV -> RoPE -> causal attn (Re Hermitian scores)
-> complex out-proj -> residual -> complex LN -> complex FFN w/ ModReLU -> residual).

Sharding over 8 NeuronCores:
  - Attention is head-sharded (16 heads -> 2 per core); LN2, out-proj, FFN and
    residuals are token-sharded (2048 tokens -> 256/core).
  - LN1 is computed REDUNDANTLY per core on all 2048 tokens (every core
    receives the full input), which deletes the hn AllGather entirely; the
    only collectives are two per-head AllToAlls (attention outputs -> token
    shards), each fired as soon as its head finishes so they overlap the
    other head's attention and the out-proj's first accumulation half.
  - The Q/K/V matmuls consume RAW fp16 x: the per-token LN affine is applied
    at PSUM eviction (per-token scale plus a rank-2 mean correction built
    from weight row-sums; for V, token-major stat scalars come from a small
    DRAM-roundtrip transpose). This takes LN1 off the QKV critical path.
  - LN gamma/beta are folded into the adjacent projection weights on the host;
    r/i complex parts are stacked in the partition dim so scores/out-proj
    contractions fuse real+imag products into single matmuls; fc1/fc2 pack
    [real | imag] moving operands into N=512 matmuls. The fc2 B stationary is
    host-negated so the f' staging needs no negation op. (An fp8-e4m3
    DoubleRow FFN variant measured the same on hardware but costs 1.6e-2 rel
    err vs 1.3e-4; it is preserved in kernel_v3_fp8.py.)
  - All logical inputs are packed into 3 dtype blobs: each extra
    ExternalInput costs ~27us/call of PJRT/axon dispatch overhead, which
    dominated the per-iteration time at 36 separate inputs.

All activations live transposed on-device: [feature, token]. All matmul
operands are fp16 (fp32 PSUM accumulation); the residual stream is fp32.
Host pre-arranges every weight in its exact SBUF layout so each weight load
is one contiguous DMA, streamed across 3 DMA queues.

Self-contained: hardcodes shapes; builds + compiles the Bass graph on first
call and runs via run_bass_kernel_spmd on cores 0-7.
"""

import contextlib
import os
import sys

sys.path.insert(0, "/opt/trn_rl_repo")

import numpy as np

import concourse.bass as bass
import concourse.bacc as bacc
import concourse.tile as tile
from concourse import mybir
from concourse.bass_utils import run_bass_kernel_spmd

# ---------------- problem dims ----------------
B, L, D, H = 2, 1024, 1024, 16
HD = D // H                  # 64
HIDDEN = 4 * D               # 4096
EPS = 1e-6
SCALE = HD ** -0.5
NC = 8                       # cores
T_ALL = B * L                # 2048 tokens
TOK = T_ALL // NC            # 256 tokens per core
KT = D // 128                # 8 k-tiles over D
HB = HIDDEN // 128           # 32 h-blocks over HIDDEN
OB = D // 128                # 8 out-blocks over D
HPC = H // NC                # 2 heads per core

F16 = mybir.dt.float16
F32 = mybir.dt.float32
F8 = mybir.dt.float8e4
AF = mybir.ActivationFunctionType
OP = mybir.AluOpType

# fp8 scaling for the FFN matmuls (e4m3: weights too small unscaled)
C1W = 1.0          # fc1 weight scale
C1T = 1.0            # f' (fc2 moving) scale
C2W = 1.0          # fc2 weight scale
DS2 = 1.0 / (C1T * C2W)  # fc2 psum descale

_cache = {}

# (name, shape, blob dtype) for every logical input, packed in this order
PACK_SPEC = [
    ("xT_r", (D, TOK), F32),
    ("xT_i", (D, TOK), F32),
    ("xT16_r", (D, T_ALL), F16),
    ("xT16_i", (D, T_ALL), F16),
    ("wq_a", (128, HPC, KT, 128), F16),
    ("wq_b", (128, HPC, KT, 128), F16),
    ("wk_a", (128, HPC, KT, 128), F16),
    ("wk_b", (128, HPC, KT, 128), F16),
    ("wv_a", (128, KT, 2 * 128), F16),
    ("wv_b", (128, KT, 2 * 128), F16),
    ("qbias", (128, HPC), F32),
    ("kbias", (128, HPC), F32),
    ("qu1", (128, HPC), F32),
    ("qu2", (128, HPC), F32),
    ("ku1", (128, HPC), F32),
    ("ku2", (128, HPC), F32),
    ("vbias_bc", (128, 2 * 128), F16),
    ("nuva_bc", (128, 2 * 128), F16),
    ("nuvb_bc", (128, 2 * 128), F16),
    ("wo_c", (128, H, D), F16),
    ("wo_d", (128, H, D), F16),
    ("obias_r", (128, OB), F32),
    ("obias_i", (128, OB), F32),
    ("w1pk", (HB, 128, KT, 2, 128), F16),
    ("bias1_r", (128, HB), F32),
    ("bias1_i", (128, HB), F32),
    ("modb", (128, HB), F32),
    ("w2pk", (OB, 128, HB, 2, 128), F16),
    ("bias2_r", (128, OB), F32),
    ("bias2_i", (128, OB), F32),
    ("bsel", (4, 512), F16),
    ("cos2", (128, L), F16),
    ("sin2", (128, L), F16),
    ("mask01", (128, 128), F16),
]


# =====================================================================
# Device kernel emission
# =====================================================================
def _emit(tc, T):
    nc = tc.nc

    with contextlib.ExitStack() as ES:
        const = ES.enter_context(tc.tile_pool(name="const", bufs=1))
        dram = ES.enter_context(tc.tile_pool(name="dramp", bufs=1, space="DRAM"))

        # ---- x16 loads FIRST: they gate LN1 stats + QK matmuls, and the
        # many small const DMAs would otherwise head-of-line block the queues
        hnp_scope = contextlib.ExitStack()
        hnp = hnp_scope.enter_context(tc.tile_pool(name="hnp", bufs=1, side="right"))
        # full-token x (fp16), consumed raw by the QK/V matmuls
        hnr_mm = [hnp.tile([128, T_ALL], F16, name=f"hnr_mm{kt}") for kt in range(KT)]
        hni_mm = [hnp.tile([128, T_ALL], F16, name=f"hni_mm{kt}") for kt in range(KT)]
        x16r_v = T["xT16_r"].rearrange("(kt p) t -> p kt t", p=128)
        x16i_v = T["xT16_i"].rearrange("(kt p) t -> p kt t", p=128)
        dmaq = [nc.sync, nc.scalar, nc.gpsimd]
        for kt in range(KT):
            dmaq[(2 * kt) % 3].dma_start(hnr_mm[kt][:], x16r_v[:, kt, :])
            dmaq[(2 * kt + 1) % 3].dma_start(hni_mm[kt][:], x16i_v[:, kt, :])

        # ---------------- constants to SBUF ----------------
        cos_sb = const.tile([128, L], F16, name="cos_sb")
        sin_sb = const.tile([128, L], F16, name="sin_sb")
        nc.sync.dma_start(cos_sb[:], T["cos2"][:])
        nc.sync.dma_start(sin_sb[:], T["sin2"][:])
        mask_sb = const.tile([128, 128], F16, name="mask_sb")
        nc.sync.dma_start(mask_sb[:], T["mask01"][:])
        ones16 = const.tile([128, 1], F16, name="ones16")
        nc.vector.memset(ones16[:], 1.0)
        ones32 = const.tile([1, 128], F32, name="ones32")
        nc.vector.memset(ones32[:], 1.0)
        qb_sb = const.tile([128, 2], F32, name="qb_sb")
        kb_sb = const.tile([128, 2], F32, name="kb_sb")
        nc.sync.dma_start(qb_sb[:], T["qbias"][:])
        nc.sync.dma_start(kb_sb[:], T["kbias"][:])
        qu1_sb = const.tile([128, HPC], F32, name="qu1_sb")
        qu2_sb = const.tile([128, HPC], F32, name="qu2_sb")
        ku1_sb = const.tile([128, HPC], F32, name="ku1_sb")
        ku2_sb = const.tile([128, HPC], F32, name="ku2_sb")
        nc.sync.dma_start(qu1_sb[:], T["qu1"][:])
        nc.sync.dma_start(qu2_sb[:], T["qu2"][:])
        nc.sync.dma_start(ku1_sb[:], T["ku1"][:])
        nc.sync.dma_start(ku2_sb[:], T["ku2"][:])
        vb_sb = const.tile([128, 2 * 128], F16, name="vb_sb")
        nc.sync.dma_start(vb_sb[:], T["vbias_bc"][:])
        nuva_bc = const.tile([128, 2 * 128], F16, name="nuva_bc")
        nuvb_bc = const.tile([128, 2 * 128], F16, name="nuvb_bc")
        nc.sync.dma_start(nuva_bc[:], T["nuva_bc"][:])
        nc.sync.dma_start(nuvb_bc[:], T["nuvb_bc"][:])
        ob_r_sb = const.tile([128, OB], F32, name="ob_r_sb")
        ob_i_sb = const.tile([128, OB], F32, name="ob_i_sb")
        nc.sync.dma_start(ob_r_sb[:], T["obias_r"][:])
        nc.sync.dma_start(ob_i_sb[:], T["obias_i"][:])
        b1r_sb = const.tile([128, HB], F32, name="b1r_sb")
        b1i_sb = const.tile([128, HB], F32, name="b1i_sb")
        modb_sb = const.tile([128, HB], F32, name="modb_sb")
        nc.sync.dma_start(b1r_sb[:], T["bias1_r"][:])
        nc.sync.dma_start(b1i_sb[:], T["bias1_i"][:])
        nc.sync.dma_start(modb_sb[:], T["modb"][:])
        b2r_sb = const.tile([128, OB], F32, name="b2r_sb")
        b2i_sb = const.tile([128, OB], F32, name="b2i_sb")
        nc.sync.dma_start(b2r_sb[:], T["bias2_r"][:])
        nc.sync.dma_start(b2i_sb[:], T["bias2_i"][:])

        # DRAM roundtrip to transpose LN1 stats into token-major [128, 48]
        # (per-token scalars for the V eviction correction)
        dram_s32 = dram.tile([4, 3, 4, 128], F32, name="dram_s32")

        # internal DRAM comm buffers (attention-out AllToAll only; LN1 is
        # computed redundantly per-core on all tokens, so no AllGather).
        # One buffer pair per local head so each head's AllToAll can fire as
        # soon as that head's outputs are normalized.
        a2a_in = [dram.tile([NC, 128, TOK], F16, name=f"a2a_in{h}")
                  for h in range(HPC)]
        a2a_out = [dram.tile([NC, 128, TOK], F16, name=f"a2a_out{h}")
                   for h in range(HPC)]

        # =====================================================
        # complex layer norm (shared by LN1 / LN2)
        #   xr/xi: [128, KT, TOK] f32 SBUF; out_fn(kt, hnr_ap, hni_ap...) style
        #   writer callbacks receive the normalized fp32 intermediates.
        # =====================================================
        def complex_ln(xr, xi, writers, lnp, lnps, tagp):
            # casts to fp16 + squares
            xr16 = lnp.tile([128, KT, TOK], F16, name=f"xr16{tagp}")
            xi16 = lnp.tile([128, KT, TOK], F16, name=f"xi16{tagp}")
            sq = lnp.tile([128, KT, TOK], F16, name=f"sq{tagp}")
            t2 = lnp.tile([128, KT, TOK], F16, name=f"t2{tagp}")
            for kt in range(KT):
                nc.vector.tensor_copy(xr16[:, kt, :], xr[:, kt, :])
                nc.vector.tensor_copy(xi16[:, kt, :], xi[:, kt, :])
                nc.scalar.activation(sq[:, kt, :], xr[:, kt, :], AF.Square)
                nc.scalar.activation(t2[:, kt, :], xi[:, kt, :], AF.Square)
                nc.vector.tensor_tensor(sq[:, kt, :], sq[:, kt, :], t2[:, kt, :], OP.add)
            # stats matmuls: sum over D (partition dim) via ones
            ps_mr = lnps.tile([1, TOK], F32, name=f"psmr{tagp}", tag=f"psmr{tagp}")
            ps_mi = lnps.tile([1, TOK], F32, name=f"psmi{tagp}", tag=f"psmi{tagp}")
            ps_sq = lnps.tile([1, TOK], F32, name=f"pssq{tagp}", tag=f"pssq{tagp}")
            for kt in range(KT):
                nc.tensor.matmul(ps_mr[:], ones16[:], xr16[:, kt, :],
                                 start=(kt == 0), stop=(kt == KT - 1))
                nc.tensor.matmul(ps_mi[:], ones16[:], xi16[:, kt, :],
                                 start=(kt == 0), stop=(kt == KT - 1))
                nc.tensor.matmul(ps_sq[:], ones16[:], sq[:, kt, :],
                                 start=(kt == 0), stop=(kt == KT - 1))
            mr = lnp.tile([1, TOK], F32, name=f"mr{tagp}")
            mi = lnp.tile([1, TOK], F32, name=f"mi{tagp}")
            msq = lnp.tile([1, TOK], F32, name=f"msq{tagp}")
            inv_d = 1.0 / D
            nc.scalar.mul(mr[:], ps_mr[:], inv_d)
            nc.scalar.mul(mi[:], ps_mi[:], inv_d)
            nc.scalar.mul(msq[:], ps_sq[:], inv_d)
            # var = msq - mr^2 - mi^2 ; rstd = exp(-0.5*ln(var+eps))
            v1 = lnp.tile([1, TOK], F32, name=f"v1{tagp}")
            nc.vector.tensor_tensor(v1[:], mr[:], mr[:], OP.mult)
            nc.vector.tensor_tensor(v1[:], msq[:], v1[:], OP.subtract)
            v2 = lnp.tile([1, TOK], F32, name=f"v2{tagp}")
            nc.vector.tensor_tensor(v2[:], mi[:], mi[:], OP.mult)
            nc.vector.tensor_tensor(v1[:], v1[:], v2[:], OP.subtract)
            nc.vector.tensor_scalar_add(v1[:], v1[:], EPS)
            rv = lnp.tile([1, TOK], F32, name=f"rv{tagp}")
            nc.scalar.activation(rv[:], v1[:], AF.Ln)
            rstd = lnp.tile([1, TOK], F32, name=f"rstd{tagp}")
            nc.scalar.activation(rstd[:], rv[:], AF.Exp, scale=-0.5)
            # broadcast mr, mi, rstd to 128 partitions via K=1 fp32 matmuls
            ps_bc = lnps.tile([128, 2 * TOK], F32, name=f"psbc{tagp}", tag=f"psbc{tagp}")
            nc.tensor.matmul(ps_bc[:, 0:TOK], ones32[:], mr[:], start=True, stop=True)
            nc.tensor.matmul(ps_bc[:, TOK:2 * TOK], ones32[:], mi[:], start=True, stop=True)
            ps_bc2 = lnps.tile([128, TOK], F32, name=f"psbc2{tagp}", tag=f"psbc2{tagp}")
            nc.tensor.matmul(ps_bc2[:], ones32[:], rstd[:], start=True, stop=True)
            bc_m = lnp.tile([128, 2 * TOK], F32, name=f"bcm{tagp}")
            bc_s = lnp.tile([128, TOK], F32, name=f"bcs{tagp}")
            nc.scalar.copy(bc_m[:], ps_bc[:])
            nc.scalar.copy(bc_s[:], ps_bc2[:])
            # normalize: hn = (x - m) * rstd  (fp16 out via writer callbacks)
            for kt in range(KT):
                tr = lnp.tile([128, TOK], F32, name=f"tr{tagp}", tag=f"tr{tagp}", bufs=2)
                nc.vector.tensor_tensor(tr[:], xr[:, kt, :], bc_m[:, 0:TOK], OP.subtract)
                ti = lnp.tile([128, TOK], F32, name=f"ti{tagp}", tag=f"ti{tagp}", bufs=2)
                nc.vector.tensor_tensor(ti[:], xi[:, kt, :], bc_m[:, TOK:2 * TOK], OP.subtract)
                writers(kt, tr, ti, bc_s)

        # =====================================================
        # Phase 2+3 scope: attention (hn computed locally for ALL tokens)
        # =====================================================
        with contextlib.ExitStack() as AS:
            attn = AS.enter_context(tc.tile_pool(name="attn", bufs=1))

            # ---- Phase 1: full-token complex LN1, redundant on every core ----
            with tc.tile_pool(name="ln1", bufs=1) as lnp, \
                 tc.tile_pool(name="ln1ps", bufs=1, space="PSUM") as lnps:
                # quarter-selector stationaries: sel4[:, 4q+j] = (j == q)
                sel4 = lnp.tile([128, 16], F16, name="sel4")
                nc.vector.memset(sel4[:], 0.0)
                for q in range(4):
                    nc.vector.memset(sel4[:, 5 * q:5 * q + 1], 1.0)
                # broadcast selector: bsel[k, 128q+p] = (k == q)  (host input:
                # single-partition memsets at partition>0 fail BIR verification)
                bsel = lnp.tile([4, 512], F16, name="bsel")
                nc.sync.dma_start(bsel[:], T["bsel"][:])

                # stats: rows = token quarters; accumulate over kt via PE
                ps_mr = lnps.tile([4, 512], F32, name="psmr1")
                ps_mi = lnps.tile([4, 512], F32, name="psmi1")
                ps_sq = lnps.tile([4, 512], F32, name="pssq1")
                for kt in range(KT):
                    sq = lnp.tile([128, T_ALL], F16, name=f"sq{kt}", tag="sq", bufs=2)
                    t2 = lnp.tile([128, T_ALL], F16, name=f"t2{kt}", tag="t2", bufs=2)
                    nc.scalar.activation(sq[:], hnr_mm[kt][:], AF.Square)
                    nc.scalar.activation(t2[:], hni_mm[kt][:], AF.Square)
                    nc.vector.tensor_tensor(sq[:], sq[:], t2[:], OP.add)
                    for q in range(4):
                        qs = slice(512 * q, 512 * (q + 1))
                        st_ = (kt == 0 and q == 0)
                        sp_ = (kt == KT - 1 and q == 3)
                        sel = sel4[:, 4 * q:4 * (q + 1)]
                        nc.tensor.matmul(ps_mr[:], sel, hnr_mm[kt][:, qs],
                                         start=st_, stop=sp_)
                        nc.tensor.matmul(ps_mi[:], sel, hni_mm[kt][:, qs],
                                         start=st_, stop=sp_)
                        nc.tensor.matmul(ps_sq[:], sel, sq[:, qs],
                                         start=st_, stop=sp_)
                # means, var, rstd on [4, 512]
                inv_d = 1.0 / D
                mr = lnp.tile([4, 512], F32, name="mr1")
                mi = lnp.tile([4, 512], F32, name="mi1")
                msq = lnp.tile([4, 512], F32, name="msq1")
                nc.scalar.mul(mr[:], ps_mr[:], inv_d)
                nc.scalar.mul(mi[:], ps_mi[:], inv_d)
                nc.scalar.mul(msq[:], ps_sq[:], inv_d)
                v1 = lnp.tile([4, 512], F32, name="v1q")
                nc.vector.tensor_tensor(v1[:], mr[:], mr[:], OP.mult)
                nc.vector.tensor_tensor(v1[:], msq[:], v1[:], OP.subtract)
                v2 = lnp.tile([4, 512], F32, name="v2q")
                nc.vector.tensor_tensor(v2[:], mi[:], mi[:], OP.mult)
                nc.vector.tensor_tensor(v1[:], v1[:], v2[:], OP.subtract)
                nc.vector.tensor_scalar_add(v1[:], v1[:], EPS)
                rv = lnp.tile([4, 512], F32, name="rvq")
                nc.scalar.activation(rv[:], v1[:], AF.Ln)
                # stats32 = [rstd | mr*rstd | mi*rstd] (f32), f16 copy for
                # the broadcast matmuls
                stats32 = lnp.tile([4, 3 * 512], F32, name="stats32")
                nc.scalar.activation(stats32[:, 0:512], rv[:], AF.Exp, scale=-0.5)
                nc.vector.tensor_tensor(stats32[:, 512:1024], mr[:],
                                        stats32[:, 0:512], OP.mult)
                nc.vector.tensor_tensor(stats32[:, 1024:1536], mi[:],
                                        stats32[:, 0:512], OP.mult)
                stats16 = lnp.tile([4, 3 * 512], F16, name="stats16")
                nc.vector.tensor_copy(stats16[:], stats32[:])
                # token-major transpose of the stats via DRAM roundtrip:
                # s16T[p, 16j + 4q + g] = stats32[q, 512j + 128g + p]
                nc.gpsimd.dma_start(
                    dram_s32.rearrange("q j g p -> q (j g p)"), stats32[:])
                s16T = attn.tile([128, 3 * 16], F32, name="s16T")
                for j in range(3):
                    for q in range(4):
                        c0 = 16 * j + 4 * q
                        nc.gpsimd.dma_start(
                            s16T[:, c0:c0 + 4],
                            dram_s32[q, j, :, :].rearrange("g p -> p g"))
                # broadcast to [128, T_ALL] via K=4 matmuls (tiles live in the
                # attn pool: QK evictions + V normalize consume them later)
                bc_s = attn.tile([128, T_ALL], F16, name="bc_s1")
                bc_msr = attn.tile([128, T_ALL], F16, name="bc_msr1")
                bc_msi = attn.tile([128, T_ALL], F16, name="bc_msi1")
                for q in range(4):
                    bq = bsel[:, 128 * q:128 * (q + 1)]
                    qs = slice(512 * q, 512 * (q + 1))
                    bps = lnps.tile([128, 3 * 512], F32, name=f"bps{q}",
                                    tag="bps", bufs=1)
                    nc.tensor.matmul(bps[:, 0:512], bq, stats16[:, 0:512],
                                     start=True, stop=True)
                    nc.tensor.matmul(bps[:, 512:1024], bq, stats16[:, 512:1024],
                                     start=True, stop=True)
                    nc.tensor.matmul(bps[:, 1024:1536], bq, stats16[:, 1024:1536],
                                     start=True, stop=True)
                    nc.scalar.copy(bc_s[:, qs], bps[:, 0:512])
                    nc.scalar.copy(bc_msr[:, qs], bps[:, 512:1024])
                    nc.scalar.copy(bc_msi[:, qs], bps[:, 1024:1536])

            # weights (host pre-arranged, one contiguous DMA each)
            wq_a = attn.tile([128, HPC, KT, 128], F16, name="wq_a")
            wq_b = attn.tile([128, HPC, KT, 128], F16, name="wq_b")
            wk_a = attn.tile([128, HPC, KT, 128], F16, name="wk_a")
            wk_b = attn.tile([128, HPC, KT, 128], F16, name="wk_b")
            for nm, t_ in (("wq_a", wq_a), ("wq_b", wq_b), ("wk_a", wk_a), ("wk_b", wk_b)):
                nc.sync.dma_start(t_[:], T[nm][:])
            wv_a = attn.tile([128, KT, 2 * 128], F16, name="wv_a")
            wv_b = attn.tile([128, KT, 2 * 128], F16, name="wv_b")
            nc.sync.dma_start(wv_a[:], T["wv_a"][:])
            nc.sync.dma_start(wv_b[:], T["wv_b"][:])

            # persistent fp16 Q/K (post-RoPE, r/i stacked per head) and V
            qbf = [attn.tile([128, T_ALL], F16, name=f"qbf{h}") for h in range(HPC)]
            kbf = [attn.tile([128, T_ALL], F16, name=f"kbf{h}") for h in range(HPC)]
            v_sb = attn.tile([128, 2 * NC, 2 * 128], F16, name="v_sb")

            def rope(dst, src, rp):
                # dst = src*cos + shift(src)*sin   (fp16 [128, 2048])
                sh = rp.tile([128, T_ALL], F16, name="sh", tag="rope_sh", bufs=2)
                for base in (0, 64):
                    nc.sync.dma_start(sh[base:base + 32, :], src[base + 32:base + 64, :])
                    nc.sync.dma_start(sh[base + 32:base + 64, :], src[base:base + 32, :])
                t1 = rp.tile([128, T_ALL], F16, name="t1", tag="rope_t1", bufs=2)
                c3 = cos_sb[:, None, :].to_broadcast((128, B, L))
                s3 = sin_sb[:, None, :].to_broadcast((128, B, L))
                src3 = src.rearrange("p (b l) -> p b l", b=B)
                sh3 = sh.rearrange("p (b l) -> p b l", b=B)
                t13 = t1.rearrange("p (b l) -> p b l", b=B)
                dst3 = dst.rearrange("p (b l) -> p b l", b=B)
                nc.vector.tensor_tensor(t13, src3, c3, OP.mult)
                nc.vector.tensor_tensor(sh3, sh3, s3, OP.mult)
                nc.vector.tensor_tensor(dst3, t13, sh3, OP.add)

            # Q/K matmuls run on RAW x (no LN dependency); the per-token LN
            # affine is applied at PSUM eviction:
            #   Q = raw·bc_s + nu1 (x) bc_msr + nu2 (x) bc_msi + bias
            # with nu1/nu2 the (negated) complex row-sums of the folded weight.
            with tc.tile_pool(name="qkps", bufs=1, space="PSUM") as qkps, \
                 tc.tile_pool(name="ropep", bufs=1) as rp:
                for hh in range(HPC):
                    for which, wa, wb, nu1, nu2, bias_col, dst in (
                            ("q", wq_a, wq_b, qu1_sb, qu2_sb,
                             qb_sb[:, hh:hh + 1], qbf[hh]),
                            ("k", wk_a, wk_b, ku1_sb, ku2_sb,
                             kb_sb[:, hh:hh + 1], kbf[hh])):
                        tmp = rp.tile([128, T_ALL], F16, name=f"tmp{which}{hh}",
                                      tag="qktmp", bufs=2)
                        ps = qkps.tile([128, T_ALL], F32, name=f"qk{which}{hh}",
                                       tag="qkps", bufs=2)
                        for kt in range(KT):
                            for ch in range(4):
                                nc.tensor.matmul(ps[:, 512 * ch:512 * (ch + 1)],
                                                 wa[:, hh, kt, :],
                                                 hnr_mm[kt][:, 512 * ch:512 * (ch + 1)],
                                                 start=(kt == 0), stop=False)
                        for kt in range(KT):
                            for ch in range(4):
                                nc.tensor.matmul(ps[:, 512 * ch:512 * (ch + 1)],
                                                 wb[:, hh, kt, :],
                                                 hni_mm[kt][:, 512 * ch:512 * (ch + 1)],
                                                 start=False, stop=(kt == KT - 1))
                        nc.vector.tensor_tensor(tmp[:], ps[:], bc_s[:], OP.mult)
                        nc.vector.scalar_tensor_tensor(
                            tmp[:], bc_msr[:], nu1[:, hh:hh + 1], tmp[:],
                            OP.mult, OP.add)
                        nc.vector.scalar_tensor_tensor(
                            tmp[:], bc_msi[:], nu2[:, hh:hh + 1], tmp[:],
                            OP.mult, OP.add)
                        rope(dst, tmp, rp)

            # V matmuls also run on RAW x; here psum is [token, feature], so
            # the LN affine uses per-token (partition) scalars from s16T and
            # per-feature negated column-sums of the packed weights:
            #   V = raw·s_t + msr_t (x) nuva + msi_t (x) nuvb + vbias
            with tc.tile_pool(name="vps_p", bufs=1, space="PSUM") as vpsp, \
                 tc.tile_pool(name="vevp", bufs=1) as vevp:
                for tt in range(2 * NC):
                    q_, g_ = tt // 4, tt % 4
                    scol = lambda j: s16T[:, 16 * j + 4 * q_ + g_:16 * j + 4 * q_ + g_ + 1]
                    vps = vpsp.tile([128, 2 * 128], F32, name=f"vps{tt}", tag="vps", bufs=4)
                    for kt in range(KT):
                        nc.tensor.matmul(vps[:], hnr_mm[kt][:, 128 * tt:128 * (tt + 1)],
                                         wv_a[:, kt, :], start=(kt == 0), stop=False)
                    for kt in range(KT):
                        nc.tensor.matmul(vps[:], hni_mm[kt][:, 128 * tt:128 * (tt + 1)],
                                         wv_b[:, kt, :], start=False, stop=(kt == KT - 1))
                    t1 = vevp.tile([128, 2 * 128], F16, name=f"vt{tt}", tag="vt", bufs=3)
                    nc.scalar.activation(t1[:], vps[:], AF.Identity, scale=scol(0))
                    nc.vector.scalar_tensor_tensor(t1[:], nuva_bc[:], scol(1), t1[:],
                                                   OP.mult, OP.add)
                    nc.vector.scalar_tensor_tensor(v_sb[:, tt, :], nuvb_bc[:], scol(2),
                                                   t1[:], OP.mult, OP.add)
            hnp_scope.close()  # free hn SBUF; lets o-proj weights prefetch

            opw_scope = contextlib.ExitStack()
            opw = opw_scope.enter_context(tc.tile_pool(name="opw", bufs=1, side="right"))
            wo_c = opw.tile([128, H, D], F16, name="wo_c")
            wo_d = opw.tile([128, H, D], F16, name="wo_d")
            nc.gpsimd.dma_start(wo_c[:], T["wo_c"][:])

            # ---------- attention core ----------
            ot_sb = [attn.tile([128, T_ALL], F16, name=f"ot_sb{h}") for h in range(HPC)]
            NB = L // 128  # 8 m-blocks per batch

            with tc.tile_pool(name="stps", bufs=1, space="PSUM") as stps, \
                 tc.tile_pool(name="otps", bufs=1, space="PSUM") as otps, \
                 tc.tile_pool(name="smps", bufs=1, space="PSUM") as smps, \
                 tc.tile_pool(name="atw", bufs=1) as atw:
                for hh in range(HPC):
                    deferred = []
                    for b in range(B):
                        t0 = L * b
                        pts = []
                        for kb in range(NB):
                            lo = 128 * kb
                            st = stps.tile([128, L], F32, name=f"st{b}{hh}{kb}",
                                           tag="st", bufs=2)
                            pieces = [(lo, 512), (512, 1024)] if lo < 512 else [(lo, 1024)]
                            for (a, e) in pieces:
                                nc.tensor.matmul(st[:, a:e],
                                                 kbf[hh][:, t0 + lo:t0 + lo + 128],
                                                 qbf[hh][:, t0 + a:t0 + e],
                                                 start=True, stop=True)
                            pt = atw.tile([128, L], F16, name=f"pt{b}{hh}{kb}",
                                          tag="pt", bufs=8)
                            nc.scalar.activation(pt[:, lo:L], st[:, lo:L], AF.Exp)
                            nc.vector.tensor_tensor(pt[:, lo:lo + 128], pt[:, lo:lo + 128],
                                                    mask_sb[:], OP.mult)
                            pts.append((kb, lo, pt))

                        ot = otps.tile([128, L], F32, name=f"ot{b}{hh}", tag="ot", bufs=1)
                        sm = smps.tile([1, L], F32, name=f"sm{b}{hh}", tag="sm", bufs=1)
                        for kb, lo, pt in pts:
                            vstat = v_sb[:, NB * b + kb, 128 * hh:128 * (hh + 1)]
                            if lo < 512:
                                pieces = [(lo, 512, kb == 0, kb == 3),
                                          (512, 1024, kb == 0, kb == NB - 1)]
                            else:
                                pieces = [(lo, 1024, False, kb == NB - 1)]
                            for (a, e, st_, sp_) in pieces:
                                nc.tensor.matmul(ot[:, a:e], vstat, pt[:, a:e],
                                                 start=st_, stop=sp_)
                        for kb, lo, pt in pts:
                            if lo < 512:
                                pieces = [(lo, 512, kb == 0, kb == 3),
                                          (512, 1024, kb == 0, kb == NB - 1)]
                            else:
                                pieces = [(lo, 1024, False, kb == NB - 1)]
                            for (a, e, st_, sp_) in pieces:
                                nc.tensor.matmul(sm[:, a:e], ones16[:], pt[:, a:e],
                                                 start=st_, stop=sp_)
                        # normalize columns by 1/rowsum (broadcast via PE)
                        rc = atw.tile([1, L], F32, name=f"rc{b}{hh}", tag="rc", bufs=4)
                        nc.vector.reciprocal(rc[:], sm[:])
                        raw = atw.tile([128, L], F16, name=f"raw{b}{hh}", tag="raw", bufs=4)
                        nc.scalar.copy(raw[:], ot[:])
                        deferred.append((b, t0, rc, raw))
                    for b, t0, rc, raw in deferred:
                        bc = stps.tile([128, L], F32, name=f"bc{b}{hh}", tag="st", bufs=2)
                        nc.tensor.matmul(bc[:, 0:512], ones32[:], rc[:, 0:512],
                                         start=True, stop=True)
                        nc.tensor.matmul(bc[:, 512:1024], ones32[:], rc[:, 512:1024],
                                         start=True, stop=True)
                        bc_sb = atw.tile([128, L], F32, name=f"bcsb{b}{hh}",
                                         tag="bcsb", bufs=2)
                        nc.scalar.copy(bc_sb[:], bc[:])
                        nc.vector.tensor_tensor(ot_sb[hh][:, t0:t0 + L], raw[:],
                                                bc_sb[:], OP.mult)
                    # this head's AllToAll fires while the next head computes
                    dstv = a2a_in[hh].rearrange("r p t -> p r t")
                    srcv = ot_sb[hh].rearrange("p (r t) -> p r t", r=NC)
                    nc.sync.dma_start(dstv[:, 0:4, :], srcv[:, 0:4, :])
                    nc.scalar.dma_start(dstv[:, 4:NC, :], srcv[:, 4:NC, :])
                    if _cache.get("no_coll"):
                        nc.sync.dma_start(a2a_out[hh].opt(), a2a_in[hh].opt())
                    else:
                        nc.gpsimd.collective_compute(
                            "AllToAll", OP.bypass,
                            replica_groups=[list(range(NC))],
                            ins=[a2a_in[hh].opt()], outs=[a2a_out[hh].opt()],
                        )

        # =====================================================
        # Phase 4: out-projection (token-parallel) + residual -> ar
        # =====================================================
        ffn = ES.enter_context(tc.tile_pool(name="ffn", bufs=1))
        ar_sb = ffn.tile([128, OB, TOK], F32, name="ar_sb")
        ai_sb = ffn.tile([128, OB, TOK], F32, name="ai_sb")

        with tc.tile_pool(name="opx", bufs=1) as opx, \
             tc.tile_pool(name="opps", bufs=2, space="PSUM") as opps:
            nc.gpsimd.dma_start(wo_d[:], T["wo_d"][:])
            og = opx.tile([128, H, TOK], F16, name="og")
            # a2a_out[s][r, p, t] -> og[p, 2r+s, t]
            ogv = og.rearrange("p (r s) t -> p r s t", s=2)
            for s in range(HPC):
                srcv = a2a_out[s].rearrange("r p t -> p r t")
                nc.sync.dma_start(ogv[:, 0:4, s, :], srcv[:, 0:4, :])
                nc.scalar.dma_start(ogv[:, 4:NC, s, :], srcv[:, 4:NC, :])
            # x^T reload for the residual
            x2r = opx.tile([128, OB, TOK], F32, name="x2r")
            x2i = opx.tile([128, OB, TOK], F32, name="x2i")
            nc.gpsimd.dma_start(x2r[:], T["xT_r"].rearrange("(kt p) t -> p kt t", p=128))
            nc.gpsimd.dma_start(x2i[:], T["xT_i"].rearrange("(kt p) t -> p kt t", p=128))
            for obk in range(OB):
                osl = slice(128 * obk, 128 * (obk + 1))
                pr = opps.tile([128, TOK], F32, name=f"pr{obk}", tag="opr", bufs=2)
                pi = opps.tile([128, TOK], F32, name=f"pi{obk}", tag="opi", bufs=2)
                # slot-0 heads (h even) arrive in the first AllToAll: accumulate
                # them first so out-proj overlaps the second collective
                horder = [2 * r for r in range(NC)] + [2 * r + 1 for r in range(NC)]
                for idx, h in enumerate(horder):
                    nc.tensor.matmul(pr[:], wo_c[:, h, osl], og[:, h, :],
                                     start=(idx == 0), stop=(idx == H - 1))
                for idx, h in enumerate(horder):
                    nc.tensor.matmul(pi[:], wo_d[:, h, osl], og[:, h, :],
                                     start=(idx == 0), stop=(idx == H - 1))
                nc.vector.scalar_tensor_tensor(ar_sb[:, obk, :], pr[:],
                                               ob_r_sb[:, obk:obk + 1], x2r[:, obk, :],
                                               OP.add, OP.add)
                nc.vector.scalar_tensor_tensor(ai_sb[:, obk, :], pi[:],
                                               ob_i_sb[:, obk:obk + 1], x2i[:, obk, :],
                                               OP.add, OP.add)
        opw_scope.close()

        # =====================================================
        # Phase 5: LN2 -> fc1 moving operand (fp8, DoubleRow pairs):
        #   mm12[:, kt, 0] = [hn2r|hn2i],  mm12[:, kt, 1] = [-hn2i|hn2r]
        # =====================================================
        mm12 = ffn.tile([128, KT, 2, 2 * TOK], F16, name="mm12")
        with tc.tile_pool(name="ln2", bufs=1) as lnp2, \
             tc.tile_pool(name="ln2ps", bufs=1, space="PSUM") as lnps2:

            def ln2_writers(kt, tr, ti, bc_s):
                m1 = mm12[:, kt, 0, :]
                m2 = mm12[:, kt, 1, :]
                nc.vector.tensor_tensor(m1[0:128, 0:TOK], tr[:], bc_s[:], OP.mult)
                nc.vector.tensor_tensor(m1[0:128, TOK:2 * TOK], ti[:], bc_s[:], OP.mult)
                nc.vector.tensor_scalar_mul(m2[0:128, 0:TOK],
                                            m1[0:128, TOK:2 * TOK], -1.0)
                nc.vector.tensor_copy(m2[0:128, TOK:2 * TOK], m1[0:128, 0:TOK])

            complex_ln(ar_sb, ai_sb, ln2_writers, lnp2, lnps2, "2")

        # =====================================================
        # Phase 6: fc1 (fp8 DoubleRow, weights x C1W) + ModReLU
        #   -> fc2 moving operand f12 (fp8, x C1T): [0]=[f'r|f'i], [1]=[-f'i|f'r]
        # =====================================================
        f12 = [ffn.tile([128, 2, 2 * TOK], F16, name=f"f12_{hb}") for hb in range(HB)]
        f2w_scope = contextlib.ExitStack()
        f2w = f2w_scope.enter_context(tc.tile_pool(name="f2w", bufs=3))
        w2l = []
        for obk in range(OB):
            w2 = f2w.tile([128, HB, 2, 128], F16, name=f"w2_{obk}", tag="w2")
            # w2 on gpsimd only: it is consumed ~100us later than w1, which
            # must not queue behind these 2.1MB blocks on sync/scalar
            nc.gpsimd.dma_start(w2[:], T["w2pk"][obk])
            w2l.append(w2)
        one_c1 = ffn.tile([128, 1], F32, name="one_c1")
        nc.vector.memset(one_c1[:], C1T / C1W)
        with tc.tile_pool(name="f1w", bufs=6) as f1w, \
             tc.tile_pool(name="mrw", bufs=4) as mrw, \
             tc.tile_pool(name="f1ps", bufs=4, space="PSUM") as f1ps:
            for hb in range(HB):
                w1 = f1w.tile([128, KT, 2, 128], F16, name=f"w1_{hb}", tag="w1")
                (nc.scalar if hb % 2 == 0 else nc.sync).dma_start(w1[:], T["w1pk"][hb])
                fps = f1ps.tile([128, 2 * TOK], F32, name=f"fps{hb}", tag="fps", bufs=6)
                for kt in range(KT):
                    nc.tensor.matmul(fps[:], w1[:, kt, 0, :], mm12[:, kt, 0, :],
                                     start=(kt == 0), stop=False)
                    nc.tensor.matmul(fps[:], w1[:, kt, 1, :], mm12[:, kt, 1, :],
                                     start=False, stop=(kt == KT - 1))
                # ModReLU (fc1 bias is structurally zero, asserted in _prep):
                # m=|f|; g=relu(1 + modb/m); f' = f*g.  fps carries C1W*fc1;
                # the Ln/Exp descale and C1T/C1W output scale fold into the
                # Relu's bias/scale constants.  f12[1] stores [f'i | -f'r] and
                # the host negates the fc2 B stationary to compensate.
                sq = mrw.tile([128, 2 * TOK], F32, name=f"sq_{hb}", tag="sq1")
                nc.scalar.activation(sq[:], fps[:], AF.Square)
                sq1 = mrw.tile([128, TOK], F32, name=f"sqs_{hb}", tag="sq2")
                nc.vector.tensor_tensor(sq1[:], sq[:, 0:TOK], sq[:, TOK:2 * TOK],
                                        OP.add)
                rs = mrw.tile([128, TOK], F32, name=f"rs_{hb}", tag="rs")
                nc.scalar.activation(rs[:], sq1[:], AF.Ln)
                rm = mrw.tile([128, TOK], F32, name=f"rm_{hb}", tag="rm")
                nc.scalar.activation(rm[:], rs[:], AF.Exp, scale=-0.5)
                # g' = (C1T/C1W)*relu(1 + modb*C1W/|z_raw|)  (modb pre-scaled)
                g = mrw.tile([128, TOK], F32, name=f"g_{hb}", tag="g")
                nc.scalar.activation(g[:], rm[:], AF.Relu, bias=one_c1[:],
                                     scale=modb_sb[:, hb:hb + 1])
                g2 = g[:, None, :].to_broadcast((128, 2, TOK))
                f0v = f12[hb][:, 0, :].rearrange("p (two t) -> p two t", two=2)
                fpsv = fps.rearrange("p (two t) -> p two t", two=2)
                nc.vector.tensor_tensor(f0v, fpsv, g2, OP.mult)
                nc.vector.tensor_copy(f12[hb][:, 1, 0:TOK], f12[hb][:, 0, TOK:2 * TOK])
                nc.vector.tensor_scalar_mul(f12[hb][:, 1, TOK:2 * TOK],
                                            f12[hb][:, 0, 0:TOK], -1.0)

        # =====================================================
        # Phase 7: fc2 + residual -> output
        #   or = w2r.f'r - w2i.f'i ; oi = w2i.f'r + w2r.f'i
        #   mm1(w2r, [f'r|f'i]) -> [or1|oi2]; mm2(w2i, [-f'i|f'r]) -> [or2|oi1]
        # =====================================================
        with tc.tile_pool(name="outp", bufs=1) as outp, \
             tc.tile_pool(name="f2ps", bufs=4, space="PSUM") as f2ps:
            for obk in range(OB):
                w2 = w2l[obk]
                ops_ = f2ps.tile([128, 2 * TOK], F32, name=f"ops{obk}", tag="ops", bufs=4)
                for hk in range(HB):
                    nc.tensor.matmul(ops_[:], w2[:, hk, 0, :], f12[hk][:, 0, :],
                                     start=(hk == 0), stop=False)
                    nc.tensor.matmul(ops_[:], w2[:, hk, 1, :], f12[hk][:, 1, :],
                                     start=False, stop=(hk == HB - 1))
                osl2 = slice(128 * obk, 128 * (obk + 1))
                # descale (psum = C1T*C2W*fc2) + bias, then residual add
                ot_r = outp.tile([128, TOK], F32, name=f"ot_r{obk}", tag="ot_r", bufs=2)
                ot_i = outp.tile([128, TOK], F32, name=f"ot_i{obk}", tag="ot_i", bufs=2)
                nc.scalar.activation(ot_r[:], ops_[:, 0:TOK], AF.Identity,
                                     bias=b2r_sb[:, obk:obk + 1], scale=DS2)
                nc.scalar.activation(ot_i[:], ops_[:, TOK:2 * TOK], AF.Identity,
                                     bias=b2i_sb[:, obk:obk + 1], scale=DS2)
                o_r = outp.tile([128, TOK], F32, name=f"o_r{obk}", tag="o_r", bufs=2)
                o_i = outp.tile([128, TOK], F32, name=f"o_i{obk}", tag="o_i", bufs=2)
                nc.vector.tensor_tensor(o_r[:], ot_r[:], ar_sb[:, obk, :], OP.add)
                nc.vector.tensor_tensor(o_i[:], ot_i[:], ai_sb[:, obk, :], OP.add)
                nc.sync.dma_start(T["outT_r"][osl2, :], o_r[:])
                nc.sync.dma_start(T["outT_i"][osl2, :], o_i[:])
        f2w_scope.close()


# =====================================================================
# Graph build + compile (cached)
# =====================================================================
def _build():
    # Bias the act-table picker toward the single set that contains every
    # func we use (Exp, Ln, Square, Relu, Identity, Copy): reorder the list so
    # that set is first (the picker takes the first covering set, so all
    # activations share one table -> one load), then remap the emitted ids
    # back to canonical act_info.json positions after compile.
    from concourse import hw_specs
    if os.environ.get("K_NO_ACTPATCH") == "1":
        _cache["act_patch"] = True
    if not _cache.get("act_patch"):
        orig = hw_specs.get_activation_tables
        PREF = "natural_log_exp_and_others"

        def reordered(arch):
            t = orig(arch)
            if PREF not in t:
                return t
            out = {PREF: t[PREF]}
            out.update({k: v for k, v in t.items() if k != PREF})
            _cache["act_names"] = (list(out.keys()), list(t.keys()))
            return out

        hw_specs.get_activation_tables = reordered
        bacc.get_activation_tables = reordered
        _cache["act_patch"] = True

    nc = bacc.Bacc("TRN2", target_bir_lowering=False, debug=False,
                   enable_asserts=False, num_devices=NC)

    # All logical inputs live inside 3 packed blobs (one per dtype): each
    # extra ExternalInput costs ~27us of per-call PJRT/axon dispatch time,
    # which dominated the per-iteration wall time at ~36 inputs.
    sizes = {F16: 0, F32: 0, F8: 0}
    offs = {}
    for name, shape, dt in PACK_SPEC:
        offs[name] = sizes[dt]
        sizes[dt] += int(np.prod(shape))
    blob = {F16: nc.dram_tensor("blob16", [sizes[F16]], F16, kind="ExternalInput"),
            F32: nc.dram_tensor("blob32", [sizes[F32]], F32, kind="ExternalInput")}
    if sizes[F8]:
        blob[F8] = nc.dram_tensor("blob8", [sizes[F8]], F8, kind="ExternalInput")
    T = {}
    for name, shape, dt in PACK_SPEC:
        n = int(np.prod(shape))
        v = blob[dt][offs[name]:offs[name] + n]
        pat_in = "(" + " ".join(f"d{i}" for i in range(len(shape))) + ")"
        pat_out = " ".join(f"d{i}" for i in range(len(shape)))
        kw = {f"d{i}": s for i, s in enumerate(shape)}
        T[name] = v.rearrange(f"{pat_in} -> {pat_out}", **kw)

    outT = nc.dram_tensor("outT", [2, D, TOK], F32, kind="ExternalOutput")
    T["outT_r"] = outT[0]
    T["outT_i"] = outT[1]

    with tile.TileContext(nc) as tc:
        _emit(tc, T)
    nc.compile()
    if "act_names" in _cache:
        reord, canon = _cache["act_names"]
        n_loads = 0
        for b in nc.main_func.blocks:
            for i in b.instructions:
                if isinstance(i, mybir.InstLoadActFuncSet):
                    i.act_func_set_id = canon.index(reord[i.act_func_set_id])
                    n_loads += 1
        _cache["n_act_loads"] = n_loads
    return nc


# =====================================================================
# Host-side input prep
# =====================================================================
def _prep(inputs):
    f32 = np.float32
    f16 = np.float16
    g1 = (np.asarray(inputs["ln1_gr"], f32) + 1j * np.asarray(inputs["ln1_gi"], f32)).astype(np.complex128)
    b1ln = (np.asarray(inputs["ln1_br"], f32) + 1j * np.asarray(inputs["ln1_bi"], f32)).astype(np.complex128)
    g2 = (np.asarray(inputs["ln2_gr"], f32) + 1j * np.asarray(inputs["ln2_gi"], f32)).astype(np.complex128)
    b2ln = (np.asarray(inputs["ln2_br"], f32) + 1j * np.asarray(inputs["ln2_bi"], f32)).astype(np.complex128)

    def cmat(r, i):
        return (np.asarray(inputs[r], f32) + 1j * np.asarray(inputs[i], f32)).astype(np.complex128)

    Wq = cmat("Wq_r", "Wq_i")
    Wk = cmat("Wk_r", "Wk_i")
    Wv = cmat("Wv_r", "Wv_i")
    Wo = cmat("Wo_r", "Wo_i")
    W1 = cmat("W1_r", "W1_i")
    W2 = cmat("W2_r", "W2_i")
    bo = (np.asarray(inputs["bo_r"], f32) + 1j * np.asarray(inputs["bo_i"], f32)).astype(np.complex128)
    b1fc = (np.asarray(inputs["b1_r"], f32) + 1j * np.asarray(inputs["b1_i"], f32)).astype(np.complex128)
    b2fc = (np.asarray(inputs["b2_r"], f32) + 1j * np.asarray(inputs["b2_i"], f32)).astype(np.complex128)
    mod_b = np.asarray(inputs["mod_b"], f32)

    Wq_e = Wq * g1[None, :] * SCALE
    Wk_e = Wk * g1[None, :]
    Wv_e = Wv * g1[None, :]
    biasQ = (Wq @ b1ln) * SCALE
    biasK = Wk @ b1ln
    biasV = Wv @ b1ln
    W1_e = W1 * g2[None, :]
    bias1 = W1 @ b2ln + b1fc

    # several ops fold these away assuming the problem's structural zeros
    assert not np.any(b1fc), "fc1 bias must be 0 (dropped in ModReLU)"
    assert not np.any(biasQ) and not np.any(biasK) and not np.any(biasV), \
        "folded QKV biases must be 0 (dropped at QK/V eviction)"

    # RoPE tables (sign-folded sin)
    inv_freq = 1.0 / (10000.0 ** (np.arange(0, HD, 2, dtype=np.float64) / HD))
    ang = np.arange(L, dtype=np.float64)[:, None] * inv_freq[None, :]
    cos_d = np.concatenate([np.cos(ang), np.cos(ang)], axis=1)
    sin_d = np.concatenate([np.sin(ang), np.sin(ang)], axis=1)
    dvec = np.arange(128) % 64
    cos2 = cos_d[:, dvec].T.astype(f16)
    sgn = np.where(dvec < 32, -1.0, 1.0)
    sin2 = (sin_d[:, dvec] * sgn[None, :]).T.astype(f16)
    mask01 = np.triu(np.ones((128, 128), dtype=f16))

    x_r = np.asarray(inputs["x_real"], f32).reshape(T_ALL, D)
    x_i = np.asarray(inputs["x_imag"], f32).reshape(T_ALL, D)

    def hsl(h):
        return slice(HD * h, HD * (h + 1))

    # fc weights packed in exact SBUF layout (shared across cores), fp8 with
    # DoubleRow K-pairs: [.., kt, 0, :] = A-part block, [.., kt, 1, :] = B-part
    f8 = f16
    w1pk = np.empty((HB, 128, KT, 2, 128), f8)
    w1rT = np.ascontiguousarray(W1_e.real.T * C1W)   # [D(k), HIDDEN]
    w1iT = np.ascontiguousarray(W1_e.imag.T * C1W)
    for hb in range(HB):
        hsl_ = slice(128 * hb, 128 * (hb + 1))
        w1pk[hb, :, :, 0] = w1rT[:, hsl_].reshape(KT, 128, 128).transpose(1, 0, 2).astype(f8)
        w1pk[hb, :, :, 1] = w1iT[:, hsl_].reshape(KT, 128, 128).transpose(1, 0, 2).astype(f8)
    # B stationary negated: f12[1] carries [f'i | -f'r]
    w2pk = np.empty((OB, 128, HB, 2, 128), f8)
    w2rT = np.ascontiguousarray(W2.real.T * C2W)     # [HIDDEN(h), D]
    w2iT = np.ascontiguousarray(W2.imag.T * (-C2W))
    for obk in range(OB):
        osl_ = slice(128 * obk, 128 * (obk + 1))
        w2pk[obk, :, :, 0] = w2rT[:, osl_].reshape(HB, 128, 128).transpose(1, 0, 2).astype(f8)
        w2pk[obk, :, :, 1] = w2iT[:, osl_].reshape(HB, 128, 128).transpose(1, 0, 2).astype(f8)

    xT16_r = np.ascontiguousarray(x_r.T.astype(f16))
    xT16_i = np.ascontiguousarray(x_i.T.astype(f16))

    maps = []
    for c in range(NC):
        m = {}
        tok = slice(TOK * c, TOK * (c + 1))
        m["xT_r"] = np.ascontiguousarray(x_r[tok].T)
        m["xT_i"] = np.ascontiguousarray(x_i[tok].T)
        m["xT16_r"] = xT16_r
        m["xT16_i"] = xT16_i

        def qk_ab(W_e):
            a = np.empty((128, HPC, KT, 128), f16)
            bb = np.empty((128, HPC, KT, 128), f16)
            for hh in range(HPC):
                h = HPC * c + hh
                A = np.concatenate([W_e.real[hsl(h), :], W_e.imag[hsl(h), :]], 0).T
                Bm = np.concatenate([-W_e.imag[hsl(h), :], W_e.real[hsl(h), :]], 0).T
                a[:, hh] = A.reshape(KT, 128, 128).transpose(1, 0, 2)
                bb[:, hh] = Bm.reshape(KT, 128, 128).transpose(1, 0, 2)
            return a, bb

        m["wq_a"], m["wq_b"] = qk_ab(Wq_e)
        m["wk_a"], m["wk_b"] = qk_ab(Wk_e)
        va = np.empty((128, KT, 2 * 128), f16)
        vb = np.empty((128, KT, 2 * 128), f16)
        vbias = np.empty(2 * 128, f32)
        for hh in range(HPC):
            h = HPC * c + hh
            A = np.concatenate([Wv_e.real[hsl(h), :], Wv_e.imag[hsl(h), :]], 0).T
            Bm = np.concatenate([-Wv_e.imag[hsl(h), :], Wv_e.real[hsl(h), :]], 0).T
            va[:, :, 128 * hh:128 * (hh + 1)] = A.reshape(KT, 128, 128).transpose(1, 0, 2)
            vb[:, :, 128 * hh:128 * (hh + 1)] = Bm.reshape(KT, 128, 128).transpose(1, 0, 2)
            vbias[128 * hh:128 * hh + 64] = biasV.real[hsl(h)]
            vbias[128 * hh + 64:128 * (hh + 1)] = biasV.imag[hsl(h)]
        m["wv_a"], m["wv_b"] = va, vb
        m["vbias_bc"] = np.tile(vbias[None, :], (128, 1)).astype(f16)
        # negated column-sums of the packed V stationaries (LN-mean correction)
        nuva = -va.astype(f32).sum(0).sum(0)    # [2*128]
        nuvb = -vb.astype(f32).sum(0).sum(0)
        m["nuva_bc"] = np.tile(nuva[None, :], (128, 1)).astype(f16)
        m["nuvb_bc"] = np.tile(nuvb[None, :], (128, 1)).astype(f16)
        qb = np.empty((128, HPC), f32)
        kb = np.empty((128, HPC), f32)
        for hh in range(HPC):
            h = HPC * c + hh
            qb[:, hh] = np.concatenate([biasQ.real[hsl(h)], biasQ.imag[hsl(h)]])
            kb[:, hh] = np.concatenate([biasK.real[hsl(h)], biasK.imag[hsl(h)]])
        m["qbias"], m["kbias"] = qb, kb

        # negated complex row-sums of the folded Q/K weights, packed [re|im]
        # (rank-2 LN-mean correction applied at QK PSUM eviction)
        def nu12(W_e):
            n1 = np.empty((128, HPC), f32)
            n2 = np.empty((128, HPC), f32)
            for hh in range(HPC):
                h = HPC * c + hh
                wr = W_e.real[hsl(h)].sum(1)
                wi = W_e.imag[hsl(h)].sum(1)
                n1[:, hh] = -np.concatenate([wr, wi])
                n2[:, hh] = -np.concatenate([-wi, wr])
            return n1, n2

        m["qu1"], m["qu2"] = nu12(Wq_e)
        m["ku1"], m["ku2"] = nu12(Wk_e)

        wo_c = np.empty((128, H, D), f16)
        wo_d = np.empty((128, H, D), f16)
        for h in range(H):
            wo_c[:, h] = np.concatenate([Wo.real[:, hsl(h)].T, -Wo.imag[:, hsl(h)].T], 0)
            wo_d[:, h] = np.concatenate([Wo.imag[:, hsl(h)].T, Wo.real[:, hsl(h)].T], 0)
        m["wo_c"], m["wo_d"] = wo_c, wo_d
        m["obias_r"] = np.ascontiguousarray(bo.real.reshape(OB, 128).T).astype(f32)
        m["obias_i"] = np.ascontiguousarray(bo.imag.reshape(OB, 128).T).astype(f32)

        m["w1pk"] = w1pk
        m["bias1_r"] = np.ascontiguousarray(bias1.real.reshape(HB, 128).T).astype(f32)
        m["bias1_i"] = np.ascontiguousarray(bias1.imag.reshape(HB, 128).T).astype(f32)
        # pre-scaled so g' = relu(C1T/C1W + (modb*C1T)/|z_raw|) = (C1T/C1W)*g
        m["modb"] = np.ascontiguousarray(
            (mod_b * C1T).reshape(HB, 128).T).astype(f32)
        m["w2pk"] = w2pk
        m["bias2_r"] = np.ascontiguousarray(b2fc.real.reshape(OB, 128).T).astype(f32)
        m["bias2_i"] = np.ascontiguousarray(b2fc.imag.reshape(OB, 128).T).astype(f32)
        bsel = np.zeros((4, 512), f16)
        for q in range(4):
            bsel[q, 128 * q:128 * (q + 1)] = 1.0
        m["bsel"] = bsel
        m["cos2"], m["sin2"], m["mask01"] = cos2, sin2, mask01
        maps.append(m)
    # pack each core's logical inputs into the 3 dtype blobs (must mirror
    # the PACK_SPEC order used at graph build time)
    packed = []
    for m in maps:
        bl = {F16: [], F32: [], F8: []}
        for name, shape, dt in PACK_SPEC:
            arr = np.ascontiguousarray(np.asarray(m[name]))
            assert arr.shape == tuple(shape), (name, arr.shape, shape)
            bl[dt].append(arr.reshape(-1))
        pm = {"blob16": np.concatenate(bl[F16]),
              "blob32": np.concatenate(bl[F32])}
        if bl[F8]:
            pm["blob8"] = np.concatenate(bl[F8])
        packed.append(pm)
    return packed


# =====================================================================
# Entry point
# =====================================================================
def kernel(**inputs):
    if os.environ.get("K_NO_COLL") == "1":
        _cache["no_coll"] = True
    if "nc" not in _cache:
        _cache["nc"] = _build()
    nc = _cache["nc"]
    in_maps = _prep(inputs)
    res = run_bass_kernel_spmd(nc, in_maps, core_ids=list(range(NC)))
    out_r = np.empty((T_ALL, D), np.float32)
    out_i = np.empty((T_ALL, D), np.float32)
    for c in range(NC):
        out_r[TOK * c:TOK * (c + 1), :] = res.results[c]["outT"][0].T
        out_i[TOK * c:TOK * (c + 1), :] = res.results[c]["outT"][1].T
    return out_r.reshape(B, L, D), out_i.reshape(B, L, D)

